# revision 20
# baseline (speedup 1.0000x reference)
"""Trainium2 Bass kernel: CTC segment-mean compression (segment_reduce).

Reference semantics (per batch element b):
  pred = argmax_V(logit)                  # softmax skipped: argmax-invariant
  segments = run-length groups of pred; padded frames excluded
  out[s, b, :] = mean of representation frames in segment s (0 if s unused)
  padding_out[b, s] = s >= num_segments(b)

Sharding: data-parallel over batch B=16 across 8 NeuronCores (2 each),
zero communication. Each core streams its 82MB logit shard (memory-bound).

Engine split for the argmax stream (the dominant phase):
  sync HWDGE queue : logit tile DMAs only (keeps the stream head short)
  GPSIMD (Pool)    : per-frame max over V (InstTensorReduce)
  Vector (DVE)     : max_index (FIND_INDEX8) + tiny ops
  scalar HWDGE queue: consts/rep/padding/output DMAs
The per-b tail (scan + one-hot matmul segment mean) runs for b=0 while
b=1's logit tiles are still streaming.
"""

import numpy as np

import concourse.bass as bass
import concourse.tile as tile
from concourse import bacc, mybir
from concourse.bass_utils import run_bass_kernel_spmd

T, B, D, V = 1024, 16, 512, 10000
NCORES = 8
BL = B // NCORES          # batch elems per core = 2
NT = T // 128             # t-tiles per batch elem = 8
BIG = 4096.0              # out-of-range segment id offset for padded frames

# The max pass uses InstTensorReduce (single-src f32 -> DVE 2x_2p mode when
# available) rather than MAX8 (1x only); max_index is the unavoidable 1x pass.

f32 = mybir.dt.float32
bf16 = mybir.dt.bfloat16
u8 = mybir.dt.uint8
u32 = mybir.dt.uint32
OP = mybir.AluOpType
AX = mybir.AxisListType
ACTF = mybir.ActivationFunctionType


def _build_nc():
    nc = bacc.Bacc()

    rep_ext = nc.declare_dram_parameter("representation", [T, BL, D], f32, isOutput=False)
    logit_ext = nc.declare_dram_parameter("logit", [T, BL, V], f32, isOutput=False)
    pad_ext = nc.declare_dram_parameter("padding", [BL, T], u8, isOutput=False)
    ident_ext = nc.declare_dram_parameter("c_ident", [128, 128], f32, isOutput=False)
    iota_ext = nc.declare_dram_parameter("c_iota", [128, T], f32, isOutput=False)
    shift_ext = nc.declare_dram_parameter("c_shift", [NT, NT], f32, isOutput=False)
    lexc_ext = nc.declare_dram_parameter("c_lexc", [NT, NT], f32, isOutput=False)
    k0_ext = nc.declare_dram_parameter("c_k0", [NT, 1], f32, isOutput=False)
    out_ext = nc.declare_dram_parameter("out", [T, BL, D], f32, isOutput=True)
    pout_ext = nc.declare_dram_parameter("pad_out", [BL, T], u8, isOutput=True)

    with tile.TileContext(nc) as tc:
        with (
            tc.tile_pool(name="const", bufs=1) as constp,
            tc.tile_pool(name="logit", bufs=3) as logitp,
            tc.tile_pool(name="mx", bufs=4) as mxp,
            tc.tile_pool(name="pred", bufs=2) as predp,
            tc.tile_pool(name="seg", bufs=2) as segp,
            tc.tile_pool(name="rep", bufs=2) as repp,
            tc.tile_pool(name="w", bufs=NT + 1) as wp,
            tc.tile_pool(name="eout", bufs=3) as eoutp,
            tc.tile_pool(name="psA", bufs=2, space="PSUM") as psA,
            tc.tile_pool(name="psB", bufs=2, space="PSUM") as psB,
        ):
            # consts + representation arrive via the scalar HWDGE queue so the
            # sync queue starts streaming logit tiles immediately.
            ident = constp.tile([128, 128], f32)
            nc.scalar.dma_start(ident[:], ident_ext[:])
            iota = constp.tile([128, T], f32)
            nc.scalar.dma_start(iota[:], iota_ext[:])
            shiftm = constp.tile([NT, NT], f32)
            nc.scalar.dma_start(shiftm[:], shift_ext[:])
            lexcm = constp.tile([NT, NT], f32)
            nc.scalar.dma_start(lexcm[:], lexc_ext[:])
            k0m = constp.tile([NT, 1], f32)
            nc.scalar.dma_start(k0m[:], k0_ext[:])
            ones_bf = constp.tile([128, 1], bf16)
            nc.vector.memset(ones_bf[:], 1.0)
            ones8 = constp.tile([128, 8], f32)
            nc.vector.memset(ones8[:], 1.0)

            repbs = []
            for b in range(BL):
                repb = repp.tile([128, NT * D], bf16, tag="repb")
                for half in range(2):
                    rep_f = repp.tile([128, NT * D // 2], f32, tag="repf")
                    nc.scalar.dma_start(
                        rep_f[:].rearrange("p (k d) -> p k d", k=NT // 2),
                        rep_ext[:, b, :].rearrange("(k p) d -> p k d", p=128)
                        [:, half * (NT // 2):(half + 1) * (NT // 2), :])
                    nc.scalar.copy(
                        repb[:, half * (NT * D // 2):(half + 1) * (NT * D // 2)],
                        rep_f[:])
                repbs.append(repb)

            prev_find = None
            for b in range(BL):
                # ---- argmax over V for this batch elem's 8 t-tiles ----
                pred_cols = predp.tile([128, NT], f32, tag="pc")
                for k in range(NT):
                    lg = logitp.tile([128, V], f32, tag="lg")
                    nc.sync.dma_start(lg[:], logit_ext[k * 128:(k + 1) * 128, b, :])
                    mx8 = mxp.tile([128, 8], f32, tag="mx8")
                    i_max = nc.vector.max(mx8[:], lg[:])
                    idx8 = mxp.tile([128, 8], u32, tag="idx")
                    i_find = nc.vector.max_index(idx8[:], mx8[:], lg[:])
                    nc.vector.tensor_copy(pred_cols[:, k:k + 1], idx8[:, 0:1])
                    # Keep the DVE strictly tile-sequential: MAX(i+1) must not
                    # be hoisted ahead of FIND(i) by the scheduler.
                    if prev_find is not None:
                        tile.add_dep_helper(i_max.ins, prev_find.ins, sync=False,
                                            reason="dve tile order")
                    prev_find = i_find

                # ---- transpose pred -> [NT, 128] (row k, col t') ----
                ps_predT = psB.tile([NT, 128], f32, tag="small")
                nc.tensor.matmul(ps_predT[:], lhsT=pred_cols[:], rhs=ident[:],
                                 start=True, stop=True)
                predT = segp.tile([NT, 128], f32, tag="predT")
                nc.scalar.copy(predT[:], ps_predT[:])

                # ---- boundary: bprev[k] = predT[k-1, 127] ----
                ps_b = psB.tile([NT, 1], f32, tag="small")
                nc.tensor.matmul(ps_b[:], lhsT=shiftm[:], rhs=predT[:, 127:128],
                                 start=True, stop=True)
                # ---- change indicators ----
                change = segp.tile([NT, 128], f32, tag="change")
                nc.vector.scalar_tensor_tensor(
                    change[:, 0:1], in0=predT[:, 0:1], scalar=ps_b[:], in1=k0m[:],
                    op0=OP.not_equal, op1=OP.mult)
                nc.vector.scalar_tensor_tensor(
                    change[:, 1:128], in0=predT[:, 1:128], scalar=0.0,
                    in1=predT[:, 0:127], op0=OP.add, op1=OP.not_equal)

                # ---- within-row inclusive cumsum ----
                segl = segp.tile([NT, 128], f32, tag="segl")
                nc.vector.tensor_tensor_scan(
                    segl[:], data0=change[:], data1=change[:], initial=0.0,
                    op0=OP.add, op1=OP.bypass)

                # ---- cross-row carry (exclusive prefix over k) ----
                ps_c = psB.tile([NT, 1], f32, tag="small")
                nc.tensor.matmul(ps_c[:], lhsT=lexcm[:], rhs=segl[:, 127:128],
                                 start=True, stop=True)
                seg0 = segp.tile([NT, 128], f32, tag="seg0")
                nc.vector.tensor_scalar(seg0[:], segl[:], ps_c[:], None, op0=OP.add)

                # ---- padding mask ----
                pad_u8 = segp.tile([NT, 128], u8, tag="padu")
                nc.sync.dma_start(
                    pad_u8[:], pad_ext[b, :].rearrange("(k t) -> k t", k=NT))
                padf = segp.tile([NT, 128], f32, tag="padf")
                i_padf = nc.vector.tensor_copy(padf[:], pad_u8[:])
                # padf is the only tail DVE op with no argmax dependency; stop
                # the scheduler from hoisting it into the DVE stream where it
                # would stall on the slow queue.
                tile.add_dep_helper(i_padf.ins, prev_find.ins, sync=False,
                                    reason="padf after argmax")
                masked = segp.tile([NT, 128], f32, tag="masked")
                nc.vector.scalar_tensor_tensor(
                    masked[:], in0=padf[:], scalar=-BIG, in1=seg0[:],
                    op0=OP.mult, op1=OP.add)
                segsel = segp.tile([NT, 128], f32, tag="segsel")
                nc.vector.scalar_tensor_tensor(
                    segsel[:], in0=padf[:], scalar=BIG, in1=seg0[:],
                    op0=OP.mult, op1=OP.add)

                # ---- new_length -> padding_out row ----
                rowmax = segp.tile([NT, 1], f32, tag="rowmax")
                nc.vector.reduce_max(rowmax[:], masked[:], axis=AX.X)
                ps_rm = psB.tile([1, NT], f32, tag="small")
                nc.tensor.matmul(ps_rm[:], lhsT=rowmax[:], rhs=ident[0:NT, 0:NT],
                                 start=True, stop=True)
                nl = segp.tile([1, 1], f32, tag="nl")   # max valid seg id (=len-1)
                nc.vector.reduce_max(nl[:], ps_rm[:], axis=AX.X)
                po = segp.tile([1, T], u8, tag="po")
                nc.vector.tensor_scalar(po[:], iota[0:1, :], nl[:], None, op0=OP.is_gt)
                nc.sync.dma_start(pout_ext[b:b + 1, :], po[:])

                # ---- transpose segsel back to [128, NT] ----
                ps_segT = psB.tile([128, NT], f32, tag="segT")
                nc.tensor.matmul(ps_segT[:], lhsT=segsel[:], rhs=ident[0:NT, 0:NT],
                                 start=True, stop=True)


                # ---- one-hot weights + segment-mean matmul ----
                repb = repbs[b]
                ws = []
                for k in range(NT):
                    w = wp.tile([128, T], bf16, tag="wk")
                    nc.vector.tensor_scalar(w[:], iota[:], ps_segT[:, k:k + 1], None,
                                            op0=OP.is_equal)
                    ws.append(w)
                for m in range(NT):
                    ps_out = psA.tile([128, D], f32, tag="big")
                    ps_cnt = psB.tile([128, 1], f32, tag="cnt")
                    for k in range(NT):
                        nc.tensor.matmul(
                            ps_out[:], lhsT=ws[k][:, m * 128:(m + 1) * 128],
                            rhs=repb[:, k * D:(k + 1) * D],
                            start=(k == 0), stop=(k == NT - 1))
                    for k in range(NT):
                        nc.tensor.matmul(
                            ps_cnt[:], lhsT=ws[k][:, m * 128:(m + 1) * 128],
                            rhs=ones_bf[:],
                            start=(k == 0), stop=(k == NT - 1))
                    cnt_cl = eoutp.tile([128, 1], f32, tag="cnt_cl")
                    nc.vector.tensor_scalar_max(cnt_cl[:], ps_cnt[:], 1.0)
                    rcp = eoutp.tile([128, 1], f32, tag="rcp")
                    nc.vector.reciprocal(rcp[:], cnt_cl[:])
                    osb = eoutp.tile([128, D], f32, tag="osb")
                    nc.scalar.activation(osb[:], ps_out[:], ACTF.Copy, scale=rcp[:])
                    nc.sync.dma_start(out_ext[m * 128:(m + 1) * 128, b, :], osb[:])

    nc.finalize()
    return nc


_NC_CACHE = None


def _consts():
    ident = np.eye(128, dtype=np.float32)
    iota = np.tile(np.arange(T, dtype=np.float32)[None, :], (128, 1))
    shift = np.eye(NT, NT, 1, dtype=np.float32)       # shift[k, m] = (m == k+1)
    kk = np.arange(NT)
    lexc = (kk[:, None] < kk[None, :]).astype(np.float32)
    k0 = (kk != 0).astype(np.float32)[:, None]
    return {
        "c_ident": ident, "c_iota": np.ascontiguousarray(iota),
        "c_shift": shift, "c_lexc": lexc, "c_k0": np.ascontiguousarray(k0),
    }


def _run(representation, logit, padding, trace=False):
    global _NC_CACHE
    if _NC_CACHE is None:
        _NC_CACHE = _build_nc()
    nc = _NC_CACHE

    rep = np.ascontiguousarray(np.asarray(representation, dtype=np.float32))
    lg = np.ascontiguousarray(np.asarray(logit, dtype=np.float32))
    pad = np.ascontiguousarray(np.asarray(padding).astype(np.uint8))
    consts = _consts()

    in_maps = []
    for i in range(NCORES):
        b0 = i * BL
        m = {
            "representation": np.ascontiguousarray(rep[:, b0:b0 + BL, :]),
            "logit": np.ascontiguousarray(lg[:, b0:b0 + BL, :]),
            "padding": np.ascontiguousarray(pad[b0:b0 + BL, :]),
        }
        m.update(consts)
        in_maps.append(m)

    res = run_bass_kernel_spmd(nc, in_maps, list(range(NCORES)), trace=trace)
    out = np.concatenate([res.results[i]["out"] for i in range(NCORES)], axis=1)
    pout = np.concatenate([res.results[i]["pad_out"] for i in range(NCORES)], axis=0)
    return (out, pout.astype(bool)), res


def kernel(representation, logit, padding):
    (out, pout), _ = _run(representation, logit, padding, trace=False)
    return out, pout


# revision 33
# speedup vs baseline: 1.0136x; 1.0136x over previous
"""Trainium2 Bass kernel: CTC segment-mean compression (segment_reduce).

Reference semantics (per batch element b):
  pred = argmax_V(logit)                  # softmax skipped: argmax-invariant
  segments = run-length groups of pred; padded frames excluded
  out[s, b, :] = mean of representation frames in segment s (0 if s unused)
  padding_out[b, s] = s >= num_segments(b)

Sharding: data-parallel over batch B=16 across 8 NeuronCores (2 each),
zero communication. Each core streams its 82MB logit shard (memory-bound).

Engine split for the argmax stream (the dominant phase, DVE-bound at
2 passes/element: MAX8 + FIND_INDEX8):
  sync HWDGE queue  : logit tile DMAs + small outputs
  Vector (DVE)      : MAX8 + max_index (the 2-pass argmax floor)
  scalar HWDGE queue: consts/representation loads (+ ACT compute)
  Scalar (ACT)      : b=0 one-hot weights (Relu(1-|s-seg|)), epilogue scale
  TensorE           : transposes, carry matmuls, segment-mean matmuls
The per-b tail (scan + one-hot matmul segment mean) for b=0 overlaps b=1's
argmax stream; PE warm-up dummies run under the final FIND so the exposed
b=1 tail matmuls run at full clock.
"""

import time

import numpy as np

import concourse.bass as bass
import concourse.tile as tile
from concourse import bacc, mybir
from concourse.bass_utils import run_bass_kernel_spmd

T, B, D, V = 1024, 16, 512, 10000
NCORES = 8
BL = B // NCORES          # batch elems per core = 2
NT = T // 128             # t-tiles per batch elem = 8
BIG = 4096.0              # out-of-range segment id offset for padded frames

f32 = mybir.dt.float32
bf16 = mybir.dt.bfloat16
u8 = mybir.dt.uint8
u32 = mybir.dt.uint32
OP = mybir.AluOpType
AX = mybir.AxisListType
ACTF = mybir.ActivationFunctionType


def _build_nc():
    nc = bacc.Bacc()

    rep_ext = nc.declare_dram_parameter("representation", [T, BL, D], f32, isOutput=False)
    logit_ext = nc.declare_dram_parameter("logit", [T, BL, V], f32, isOutput=False)
    pad_ext = nc.declare_dram_parameter("padding", [BL, T], u8, isOutput=False)
    ident_ext = nc.declare_dram_parameter("c_ident", [128, 128], f32, isOutput=False)
    iota_ext = nc.declare_dram_parameter("c_iota", [128, T], f32, isOutput=False)
    shift_ext = nc.declare_dram_parameter("c_shift", [NT, NT], f32, isOutput=False)
    lexc_ext = nc.declare_dram_parameter("c_lexc", [NT, NT], f32, isOutput=False)
    k0_ext = nc.declare_dram_parameter("c_k0", [NT, 1], f32, isOutput=False)
    nident_ext = nc.declare_dram_parameter("c_nident", [NT, NT], f32, isOutput=False)
    out_ext = nc.declare_dram_parameter("out", [T, BL, D], f32, isOutput=True)
    pout_ext = nc.declare_dram_parameter("pad_out", [BL, T], u8, isOutput=True)

    with tile.TileContext(nc) as tc:
        with (
            tc.tile_pool(name="const", bufs=1) as constp,
            tc.tile_pool(name="logit", bufs=3) as logitp,
            tc.tile_pool(name="mx", bufs=4) as mxp,
            tc.tile_pool(name="pred", bufs=2) as predp,
            tc.tile_pool(name="seg", bufs=2) as segp,
            tc.tile_pool(name="rep", bufs=2) as repp,
            tc.tile_pool(name="w", bufs=NT + 1) as wp,
            tc.tile_pool(name="wd", bufs=2) as wdp,
            tc.tile_pool(name="eout", bufs=3) as eoutp,
            tc.tile_pool(name="psA", bufs=2, space="PSUM") as psA,
            tc.tile_pool(name="psB", bufs=2, space="PSUM") as psB,
        ):
            # consts + representation arrive via the scalar HWDGE queue so the
            # sync queue starts streaming logit tiles immediately.
            ident = constp.tile([128, 128], f32)
            nc.scalar.dma_start(ident[:], ident_ext[:])
            iota = constp.tile([128, T], f32)
            nc.scalar.dma_start(iota[:], iota_ext[:])
            shiftm = constp.tile([NT, NT], f32)
            nc.scalar.dma_start(shiftm[:], shift_ext[:])
            lexcm = constp.tile([NT, NT], f32)
            nc.scalar.dma_start(lexcm[:], lexc_ext[:])
            k0m = constp.tile([NT, 1], f32)
            nc.scalar.dma_start(k0m[:], k0_ext[:])
            ones_bf = constp.tile([128, 1], bf16)
            nc.vector.memset(ones_bf[:], 1.0)
            ones8 = constp.tile([128, 8], f32)
            nc.vector.memset(ones8[:], 1.0)
            nident = constp.tile([NT, NT], f32)
            nc.scalar.dma_start(nident[:], nident_ext[:])

            repbs = []
            for b in range(BL):
                repb = repp.tile([128, NT * D], bf16, tag="repb")
                for half in range(2):
                    rep_f = repp.tile([128, NT * D // 2], f32, tag="repf")
                    nc.scalar.dma_start(
                        rep_f[:].rearrange("p (k d) -> p k d", k=NT // 2),
                        rep_ext[:, b, :].rearrange("(k p) d -> p k d", p=128)
                        [:, half * (NT // 2):(half + 1) * (NT // 2), :])
                    nc.scalar.copy(
                        repb[:, half * (NT * D // 2):(half + 1) * (NT * D // 2)],
                        rep_f[:])
                repbs.append(repb)

            prev_find = None
            for b in range(BL):
                # ---- argmax over V for this batch elem's 8 t-tiles ----
                pred_cols = predp.tile([128, NT], f32, tag="pc")
                for k in range(NT):
                    lg = logitp.tile([128, V], f32, tag="lg")
                    nc.sync.dma_start(lg[:], logit_ext[k * 128:(k + 1) * 128, b, :])
                    mx8 = mxp.tile([128, 8], f32, tag="mx8")
                    i_max = nc.vector.max(mx8[:], lg[:])
                    idx8 = mxp.tile([128, 8], u32, tag="idx")
                    i_find = nc.vector.max_index(idx8[:], mx8[:], lg[:])
                    nc.vector.tensor_copy(pred_cols[:, k:k + 1], idx8[:, 0:1])
                    # Keep the DVE strictly tile-sequential: MAX(i+1) must not
                    # be hoisted ahead of FIND(i) by the scheduler.
                    if prev_find is not None:
                        tile.add_dep_helper(i_max.ins, prev_find.ins, sync=False,
                                            reason="dve tile order")
                    prev_find = i_find

                if b == BL - 1:
                    # Warm the TensorEngine out of its cold p-state during the
                    # final FIND (PE is idle), so the tail matmuls run at full
                    # clock. ~3us of continuous dummy work promotes the HAM.
                    ps_dummy = psA.tile([128, D], f32, tag="big")
                    for _ in range(14):
                        i_dm = nc.tensor.matmul(
                            ps_dummy[:], lhsT=repbs[b][:, 0:128],
                            rhs=repbs[b][:, 0:D], start=True, stop=True)
                        tile.add_dep_helper(i_dm.ins, i_max.ins, sync=False,
                                            reason="pe warmup after last max")

                # ---- transpose pred -> [NT, 128] (row k, col t') ----
                ps_predT = psB.tile([NT, 128], f32, tag="small")
                nc.tensor.matmul(ps_predT[:], lhsT=pred_cols[:], rhs=ident[:],
                                 start=True, stop=True)
                predT = segp.tile([NT, 128], f32, tag="predT")
                nc.scalar.copy(predT[:], ps_predT[:])

                # ---- boundary: bprev[k] = predT[k-1, 127] ----
                ps_b = psB.tile([NT, 1], f32, tag="small")
                nc.tensor.matmul(ps_b[:], lhsT=shiftm[:], rhs=predT[:, 127:128],
                                 start=True, stop=True)
                # ---- change indicators ----
                change = segp.tile([NT, 128], f32, tag="change")
                nc.vector.scalar_tensor_tensor(
                    change[:, 0:1], in0=predT[:, 0:1], scalar=ps_b[:], in1=k0m[:],
                    op0=OP.not_equal, op1=OP.mult)
                nc.vector.scalar_tensor_tensor(
                    change[:, 1:128], in0=predT[:, 1:128], scalar=0.0,
                    in1=predT[:, 0:127], op0=OP.add, op1=OP.not_equal)

                # ---- within-row inclusive cumsum ----
                segl = segp.tile([NT, 128], f32, tag="segl")
                nc.vector.tensor_tensor_scan(
                    segl[:], data0=change[:], data1=change[:], initial=0.0,
                    op0=OP.add, op1=OP.bypass)

                # ---- cross-row carry (exclusive prefix over k) ----
                ps_c = psB.tile([NT, 1], f32, tag="small")
                nc.tensor.matmul(ps_c[:], lhsT=lexcm[:], rhs=segl[:, 127:128],
                                 start=True, stop=True)
                seg0 = segp.tile([NT, 128], f32, tag="seg0")
                nc.vector.tensor_scalar(seg0[:], segl[:], ps_c[:], None, op0=OP.add)

                # ---- padding mask ----
                pad_u8 = segp.tile([NT, 128], u8, tag="padu")
                nc.sync.dma_start(
                    pad_u8[:], pad_ext[b, :].rearrange("(k t) -> k t", k=NT))
                padf = segp.tile([NT, 128], f32, tag="padf")
                i_padf = nc.vector.tensor_copy(padf[:], pad_u8[:])
                # padf is the only tail DVE op with no argmax dependency; stop
                # the scheduler from hoisting it into the DVE stream where it
                # would stall on the slow queue.
                tile.add_dep_helper(i_padf.ins, prev_find.ins, sync=False,
                                    reason="padf after argmax")
                masked = segp.tile([NT, 128], f32, tag="masked")
                nc.vector.scalar_tensor_tensor(
                    masked[:], in0=padf[:], scalar=-BIG, in1=seg0[:],
                    op0=OP.mult, op1=OP.add)
                segsel = segp.tile([NT, 128], f32, tag="segsel")
                nc.vector.scalar_tensor_tensor(
                    segsel[:], in0=padf[:], scalar=BIG, in1=seg0[:],
                    op0=OP.mult, op1=OP.add)

                # ---- new_length -> padding_out row ----
                rowmax = segp.tile([NT, 1], f32, tag="rowmax")
                nc.vector.reduce_max(rowmax[:], masked[:], axis=AX.X)
                ps_rm = psB.tile([1, NT], f32, tag="small")
                nc.tensor.matmul(ps_rm[:], lhsT=rowmax[:], rhs=ident[0:NT, 0:NT],
                                 start=True, stop=True)
                nl = segp.tile([1, 1], f32, tag="nl")   # max valid seg id (=len-1)
                nc.vector.reduce_max(nl[:], ps_rm[:], axis=AX.X)
                po = segp.tile([1, T], u8, tag="po")
                nc.vector.tensor_scalar(po[:], iota[0:1, :], nl[:], None, op0=OP.is_gt)
                nc.sync.dma_start(pout_ext[b:b + 1, :], po[:])

                # ---- transpose segsel back to [128, NT] ----
                # b=0 runs mid-stream: one-hot built on the idle Scalar engine
                # as W = Relu(1 - |s - seg|) (exact on small ints), keeping the
                # DVE free for the argmax stream. b=1 runs in the tail where
                # the DVE is idle and faster (is_equal at 2x).
                on_act = (b == 0)
                ps_segT = psB.tile([128, NT], f32, tag="segT")
                nc.tensor.matmul(ps_segT[:], lhsT=segsel[:],
                                 rhs=(nident if on_act else ident)[0:NT, 0:NT],
                                 start=True, stop=True)

                repb = repbs[b]
                ws = []
                if on_act:
                    segTn = segp.tile([128, NT], f32, tag="segTn")
                    nc.scalar.copy(segTn[:], ps_segT[:])
                    for k in range(NT):
                        wd = wdp.tile([128, T], f32, tag="wd")
                        nc.scalar.activation(wd[:], iota[:], ACTF.Abs,
                                             bias=segTn[:, k:k + 1], scale=1.0)
                        w = wp.tile([128, T], bf16, tag="wk")
                        nc.scalar.activation(w[:], wd[:], ACTF.Relu,
                                             bias=1.0, scale=-1.0)
                        ws.append(w)
                else:
                    for k in range(NT):
                        w = wp.tile([128, T], bf16, tag="wk")
                        nc.vector.tensor_scalar(w[:], iota[:], ps_segT[:, k:k + 1],
                                                None, op0=OP.is_equal)
                        ws.append(w)
                for m in range(NT):
                    wm = [ws[k][:, m * 128:(m + 1) * 128] for k in range(NT)]
                    ps_out = psA.tile([128, D], f32, tag="big")
                    ps_cnt = psB.tile([128, 1], f32, tag="cnt")
                    for k in range(NT):
                        nc.tensor.matmul(
                            ps_out[:], lhsT=wm[k],
                            rhs=repb[:, k * D:(k + 1) * D],
                            start=(k == 0), stop=(k == NT - 1))
                    for k in range(NT):
                        nc.tensor.matmul(
                            ps_cnt[:], lhsT=wm[k],
                            rhs=ones_bf[:],
                            start=(k == 0), stop=(k == NT - 1))
                    cnt_cl = eoutp.tile([128, 1], f32, tag="cnt_cl")
                    nc.vector.tensor_scalar_max(cnt_cl[:], ps_cnt[:], 1.0)
                    rcp = eoutp.tile([128, 1], f32, tag="rcp")
                    nc.vector.reciprocal(rcp[:], cnt_cl[:])
                    osb = eoutp.tile([128, D], f32, tag="osb")
                    nc.scalar.activation(osb[:], ps_out[:], ACTF.Copy, scale=rcp[:])
                    nc.sync.dma_start(out_ext[m * 128:(m + 1) * 128, b, :], osb[:])

    nc.finalize()
    return nc


_NC_CACHE = None


def _consts():
    ident = np.eye(128, dtype=np.float32)
    iota = np.tile(np.arange(T, dtype=np.float32)[None, :], (128, 1))
    shift = np.eye(NT, NT, 1, dtype=np.float32)       # shift[k, m] = (m == k+1)
    kk = np.arange(NT)
    lexc = (kk[:, None] < kk[None, :]).astype(np.float32)
    k0 = (kk != 0).astype(np.float32)[:, None]
    return {
        "c_ident": ident, "c_iota": np.ascontiguousarray(iota),
        "c_shift": shift, "c_lexc": lexc, "c_k0": np.ascontiguousarray(k0),
        "c_nident": -np.eye(NT, dtype=np.float32),
    }


def _run(representation, logit, padding, trace=False):
    global _NC_CACHE
    if _NC_CACHE is None:
        _NC_CACHE = _build_nc()
    nc = _NC_CACHE

    rep = np.ascontiguousarray(np.asarray(representation, dtype=np.float32))
    lg = np.ascontiguousarray(np.asarray(logit, dtype=np.float32))
    pad = np.ascontiguousarray(np.asarray(padding).astype(np.uint8))
    consts = _consts()

    in_maps = []
    for i in range(NCORES):
        b0 = i * BL
        m = {
            "representation": np.ascontiguousarray(rep[:, b0:b0 + BL, :]),
            "logit": np.ascontiguousarray(lg[:, b0:b0 + BL, :]),
            "padding": np.ascontiguousarray(pad[b0:b0 + BL, :]),
        }
        m.update(consts)
        in_maps.append(m)

    res = None
    last_err = None
    for attempt in range(3):
        try:
            res = run_bass_kernel_spmd(nc, in_maps, list(range(NCORES)), trace=trace)
            break
        except Exception as e:  # transient NRT device errors recover on retry
            last_err = e
            time.sleep(5)
    if res is None:
        raise last_err
    out = np.concatenate([res.results[i]["out"] for i in range(NCORES)], axis=1)
    pout = np.concatenate([res.results[i]["pad_out"] for i in range(NCORES)], axis=0)
    return (out, pout.astype(bool)), res


def kernel(representation, logit, padding):
    (out, pout), _ = _run(representation, logit, padding, trace=False)
    return out, pout
